# revision 33
# baseline (speedup 1.0000x reference)
"""Bidirectional attention kernel for Trainium2 (Bass/Tile), SPMD over 8 cores.

Per batch n (one batch per core):
    scores  = (lhs * w_lhs) @ (rhs * w_rhs).T          # [L, R]
            = (lhs * (w_lhs*w_rhs)) @ rhs.T            # diagonal scales compose
    E       = exp(scores)                              # no max-subtraction:
                                                       # |scores| < ~0.5 here
    lhs_ctx = (E @ rhs)   / rowsum(E)                  # row softmax folded into scale
    rhs_ctx = (E.T @ lhs) / colsum(E)                  # col softmax folded into scale
    out_lhs = [lhs | lhs_ctx],  out_rhs = [rhs | rhs_ctx]

Dual-S design: S^T is computed directly on the tensor engine from the same
resident fp8 operands instead of XBAR-transposing exp(S) (+33% PE work,
which has headroom; kills the serial Exp->transpose->cast chain and 8 MB of
SBUF<->SBUF DMA). Exp writes fp8 E and E^T directly; its accum_out yields
the row/col sums for free. All matmuls are fp8 DoubleRow (contraction
256/instr, FD=512): 1024 MMs/core, ~140us PE floor.

fp8 range: T1 carries 16*(w_lhs*w_rhs) to lift lhs*w2 (~1e-2) out of the
fp8e4 subnormal zone; the Exp activation applies scale=1/16 to compensate.

Asymmetric streaming schedule (per-engine queues execute IN ORDER, so the
emission order is the schedule):
1. lhs streams first (bf16 cast-DMA loads -> transpose -> fp8 casts),
   ~30us with PE idle -- the unavoidable head.
2. rhs streams; as chunk k lands, the FULL S^T row k runs (its moving
   operand, all of T1, is ready) -- PE works through the entire rhs
   stream. First half of rhs loads as f32 so the raw out_rhs half can be
   written from SBUF inside this window's DMA slack.
3. Natural-S row i + lhs_ctx row i (C1) interleave: C1 needs complete E^T
   (step 2) + rowsum(i) which natural row i just produced.
4. rhs_ctx rows (C2) close; DRAM->DRAM raw-half concats (lhs + second
   half of rhs) spread through steps 3-4 on the then-idle SWDGE queue.

E layouts are pair-blocked [tok%128, chunk, pair, 2, tok%128] so ctx
stationaries are contiguous 256B DoubleRow pair-blocks."""

import os
import sys

import numpy as np

for _p in ("/root/.axon_site/_ro/trn_rl_repo", "/opt/trn_rl_repo"):
    if os.path.isdir(_p) and _p not in sys.path:
        sys.path.append(_p)

N_CORES = 8
L, R, D = 2048, 2048, 1024


def build_program(L, R, D, repeat=1, phases="all"):
    from contextlib import ExitStack

    import concourse.bass as bass
    import concourse.mybir as mybir
    import concourse.tile as tile
    from concourse import bacc

    f32 = mybir.dt.float32
    bf16 = mybir.dt.bfloat16
    f8 = mybir.dt.float8e4
    DR = mybir.MatmulPerfMode.DoubleRow
    P = 128
    LC, RC, DC = L // P, R // P, D // P
    assert DC % 2 == 0 and RC % 2 == 0 and LC % 2 == 0
    QW = min(512, R)        # moving-operand width per matmul (scores FD)
    EW = min(1024, R)       # Exp width (psum tile, 2 banks)
    NH = R // EW            # Exp tiles per chunk-row
    EC = EW // P            # chunks per Exp tile
    QH = EW // QW           # QW-tiles per Exp tile
    MN = min(512, D)        # ctx matmul moving free width
    NMN = D // MN
    RSB = 0                 # all raw halves go DRAM->DRAM (keeps the rin f32
                            # path out of the stream; SBUF goes to staging)
    CH = min(8, LC)         # chunks per transpose group (amortizes the
                            # transpose-vs-DMA serialization window)

    nc = bacc.Bacc("TRN2", target_bir_lowering=False, debug=False)

    lhs = nc.dram_tensor("lhs", [L, D], f32, kind="ExternalInput")
    rhs = nc.dram_tensor("rhs", [R, D], f32, kind="ExternalInput")
    w_lhs = nc.dram_tensor("w_lhs", [1, D], f32, kind="ExternalInput")
    w_rhs = nc.dram_tensor("w_rhs", [1, D], f32, kind="ExternalInput")
    out_lhs = nc.dram_tensor("out_lhs", [L, 2 * D], f32, kind="ExternalOutput")
    out_rhs = nc.dram_tensor("out_rhs", [R, 2 * D], f32, kind="ExternalOutput")

    Exp = mybir.ActivationFunctionType.Exp
    Copy = mybir.ActivationFunctionType.Copy
    mult = mybir.AluOpType.mult
    add = mybir.AluOpType.add

    with tile.TileContext(nc) as tc, ExitStack() as ctx:
        const = ctx.enter_context(tc.tile_pool(name="const", bufs=1))
        res = ctx.enter_context(tc.tile_pool(name="res", bufs=1))
        tbp = ctx.enter_context(tc.tile_pool(name="tbp", bufs=2))
        ttp = ctx.enter_context(tc.tile_pool(name="ttp", bufs=2))
        outp = ctx.enter_context(tc.tile_pool(name="outp", bufs=3))
        scal = ctx.enter_context(tc.tile_pool(name="scal", bufs=4))

        # Resident fp8 transposed operands, [d%128, d//128, tok]: moving
        # slices merge to 3D [128, 2, QW]; stationary slices are [128, 2,
        # 128] with pair stride L.
        T1D = res.tile([P, DC, L], f8, tag="T1D")
        T2D = res.tile([P, DC, R], f8, tag="T2D")
        # E matrices, pair-blocked so ctx stationaries are contiguous:
        # Ef8[l%128, r_chunk, l_pair, l_parity, r%128] = exp(S)[l, r]
        Ef8 = res.tile([P, RC, LC // 2, 2, P], f8, tag="Ef8")
        ETf8 = res.tile([P, LC, RC // 2, 2, P], f8, tag="ETf8")
        lhsb8 = res.tile([P, LC, D], f8, tag="lhsb8")
        rhsb8 = res.tile([P, RC, D], f8, tag="rhsb8")
        rsum = res.tile([P, LC, NH], f32, tag="rsum")
        csum = res.tile([P, RC, NH], f32, tag="csum")

        psS = ctx.enter_context(tc.tile_pool(name="psS", bufs=2, space="PSUM"))
        psC = ctx.enter_context(tc.tile_pool(name="psC", bufs=2, space="PSUM"))

        for rep in range(repeat):
            # w2 = 16 * w_lhs * w_rhs in [d%128, d//128] layout
            wlT = const.tile([P, DC], f32, tag="wlT")
            wrT = const.tile([P, DC], f32, tag="wrT")
            w2T = const.tile([P, DC], f32, tag="w2T")
            # w layout loads as DC small partition-spray DMAs (512B
            # contiguous each) on the ACT HWDGE ring -- a single rearranged
            # gather would be 1024 tiny descriptors occupying the DMA device
            # for ~10us, which every XBAR transpose serializes against.
            for wt, wsrc in ((wlT, w_lhs), (wrT, w_rhs)):
                for dc in range(DC):
                    nc.scalar.dma_start(
                        wt[:, dc:dc + 1],
                        wsrc[0, dc * P:(dc + 1) * P].rearrange(
                            "(di one) -> di one", one=1),
                    )

            def emit_w2():
                # emitted AFTER the lhs stream: the gathers above finish
                # under it, so nothing ever waits on w2T at a queue head
                nc.vector.tensor_mul(w2T[:], wlT[:], wrT[:])
                nc.scalar.activation(w2T[:], w2T[:], Copy, scale=16.0)

            TW = min(2, CH)     # chunks per transpose (2D [128, TW*D] source,
                                # the baseline-proven wide shape)

            def prep_l_loads(g):
                c0 = g * CH
                tb4 = tbp.tile([P, CH * D], bf16, tag="tb4", name=f"tb4_l{g}")
                for j in range(CH):
                    c = c0 + j
                    nc.gpsimd.dma_start(tb4[:, j * D:(j + 1) * D],
                                        lhs[c * P:(c + 1) * P, :])
                return tb4

            def prep_l_group(g, emit_rows=False, after_pair0=None):
                c0 = g * CH
                tb4 = prep_l_loads(g)
                tT4 = ttp.tile([P, CH, DC, P], bf16, tag="tT4", name=f"tT4_l{g}")
                for j0 in range(0, CH, TW):
                    nc.sync.dma_start_transpose(
                        tT4[:, j0:j0 + TW, :, :], tb4[:, j0 * D:(j0 + TW) * D])
                w2b = w2T[:, :, None].to_broadcast((P, DC, P))
                for j0 in range(0, CH, TW):
                    for j in range(j0, j0 + TW):
                        c = c0 + j
                        nc.vector.tensor_tensor(T1D[:, :, c * P:(c + 1) * P],
                                                tT4[:, j, :, :], w2b, mult)
                        nc.vector.tensor_copy(lhsb8[:, c, :],
                                              tb4[:, j * D:(j + 1) * D])
                    if emit_rows:
                        for j in range(j0, j0 + TW):
                            score_half(T1D, T2D, Ef8, rsum, c0 + j, 0)
                    if j0 == 0 and after_pair0 is not None:
                        after_pair0()

            def prep_r_loads(g):
                c0 = g * CH
                tb4 = tbp.tile([P, CH * D], bf16, tag="tb4", name=f"tb4_r{g}")
                for j in range(CH):
                    c = c0 + j
                    nc.gpsimd.dma_start(tb4[:, j * D:(j + 1) * D],
                                        rhs[c * P:(c + 1) * P, :])
                return tb4

            def prep_r_casts(g, tb4):
                c0 = g * CH
                tT4 = ttp.tile([P, CH, DC, P], bf16, tag="tT4", name=f"tT4_r{g}")
                for j in range(0, CH, TW):
                    nc.sync.dma_start_transpose(
                        tT4[:, j:j + TW, :, :], tb4[:, j * D:(j + TW) * D])
                for j in range(CH):
                    c = c0 + j
                    nc.vector.tensor_copy(T2D[:, :, c * P:(c + 1) * P],
                                          tT4[:, j, :, :])
                    nc.vector.tensor_copy(rhsb8[:, c, :], tb4[:, j * D:(j + 1) * D])
                return tT4

            def prep_r_group(g):
                return prep_r_casts(g, prep_r_loads(g))

            def score_half(stat_D, mov_D, out_e, out_sum, c, h):
                """Half-row (c, h) of scores + Exp into the blocked E layout."""
                ps = psS.tile([P, EW], f32, tag="psS", name=f"ps_{c}_{h}")
                for dcp in range(0, DC, 2):
                    for q in range(QH):
                        t = h * QH + q
                        nc.tensor.matmul(
                            ps[:, q * QW:(q + 1) * QW],
                            stat_D[:, dcp:dcp + 2, c * P:(c + 1) * P],
                            mov_D[:, dcp:dcp + 2, t * QW:(t + 1) * QW],
                            start=(dcp == 0), stop=(dcp == DC - 2),
                            perf_mode=DR,
                        )
                nc.scalar.activation(
                    out_e[:, h * EC:(h + 1) * EC, c // 2, c % 2, :],
                    ps[:], Exp, scale=0.0625,
                    accum_out=out_sum[:, c, h:h + 1],
                )

            def ctx_row(stat_e, mov_nat, sums, out, CN, c, pool, lbl):
                """ctx row c: (E-slice @ mov) / sum -> out[c-chunk, D:2D]."""
                pc = pool.tile([P, D], f32, tag=pool.name, name=f"pc_{lbl}_{c}")
                for kp in range(0, CN, 2):
                    for q in range(NMN):
                        nc.tensor.matmul(
                            pc[:, q * MN:(q + 1) * MN],
                            stat_e[:, c, kp // 2, :, :],
                            mov_nat[:, kp:kp + 2, q * MN:(q + 1) * MN],
                            start=(kp == 0), stop=(kp == CN - 2),
                            perf_mode=DR,
                        )
                tot = scal.tile([P, 1], f32, tag="tot", name=f"tot_{lbl}{c}")
                rec = scal.tile([P, 1], f32, tag="rec", name=f"rec_{lbl}{c}")
                nc.vector.tensor_reduce(tot[:], sums[:, c, :], mybir.AxisListType.X, add)
                nc.vector.reciprocal(rec[:], tot[:])
                co = outp.tile([P, D], f32, tag="ctxo", name=f"co_{lbl}{c}")
                nc.vector.tensor_scalar_mul(co[:], pc[:], rec[:])
                nc.sync.dma_start(out[c * P:(c + 1) * P, D:2 * D], co[:])

            # DRAM->DRAM raw-half concats (all lhs + second half of rhs),
            # spread one per back-phase row on the then-idle SWDGE queue.
            concats = [(out_lhs, lhs, c) for c in range(LC)] + [
                (out_rhs, rhs, c) for c in range(RSB, RC)
            ]

            def pop_concat():
                # on the SP ring: the ctx-write FIFO ahead of each concat
                # paces it into the back phases (on the dep-free Pool queue
                # they would all fire during the input streams)
                if concats:
                    out, src, c = concats.pop(0)
                    nc.sync.dma_start(
                        out[c * P:(c + 1) * P, 0:D], src[c * P:(c + 1) * P, :]
                    )

            # 1. rhs head (chunks 0..EC-1: everything natural-h0 reads),
            # then lhs streams in transpose-pairs, each unlocking its
            # natural-S h=0 rows -- PE starts after ~EC+2 chunk loads.
            NLG, NRG = LC // CH, RC // CH
            RHEAD = max(1, EC // CH)
            tT4_rh = None
            for g in range(RHEAD):
                tT4_rh = prep_r_group(g)
            emit_w2()
            # Pool-queue barrier: a tiny gpsimd op depending on the last
            # rhs-head transpose holds the lhs loads back so they never
            # interleave with (and serialize against) those transposes.
            bar = scal.tile([P, 1], f32, tag="bar", name="bar")
            nc.gpsimd.tensor_copy(bar[:], tT4_rh[:, CH - 1, DC - 1, 0:1])
            wave2 = {}

            def emit_wave2_loads():
                if NLG > 1:
                    wave2["l1"] = prep_l_loads(1)
                for rg in range(RHEAD, NRG):
                    wave2[rg] = prep_r_loads(rg)

            prep_l_group(0, emit_rows=True, after_pair0=emit_wave2_loads)
            # S^T h=0 rows for the rhs head: need only lhs group 0 (T1
            # quarters 0-1) + the head's T2D -- extra PE backlog that covers
            # the second-wave prep window.
            for k in range(min(CH, RC)):
                score_half(T2D, T1D, ETf8, csum, k, 0)
            if NLG > 1:
                # lhs g1 casts + its h=0 rows first (they gate PE soonest)
                tb4 = wave2.pop("l1")
                tT4 = ttp.tile([P, CH, DC, P], bf16, tag="tT4", name="tT4_l1")
                for j0 in range(0, CH, TW):
                    nc.sync.dma_start_transpose(
                        tT4[:, j0:j0 + TW, :, :], tb4[:, j0 * D:(j0 + TW) * D])
                w2b = w2T[:, :, None].to_broadcast((P, DC, P))
                for j0 in range(0, CH, TW):
                    for j in range(j0, j0 + TW):
                        c = CH + j
                        nc.vector.tensor_tensor(T1D[:, :, c * P:(c + 1) * P],
                                                tT4[:, j, :, :], w2b, mult)
                        nc.vector.tensor_copy(lhsb8[:, c, :],
                                              tb4[:, j * D:(j + 1) * D])
                    for j in range(j0, j0 + TW):
                        score_half(T1D, T2D, Ef8, rsum, CH + j, 0)
                if NH > 1:
                    # S^T h=1 rows of the head (T1 quarters 2-3 just landed)
                    for k in range(min(CH, RC)):
                        score_half(T2D, T1D, ETf8, csum, k, 1)
            for rg in range(RHEAD, NRG):
                prep_r_casts(rg, wave2.pop(rg))
            # 2. natural-S h=1 rows (rhs tail is in by now)
            for h in range(1, NH):
                for i in range(LC):
                    score_half(T1D, T2D, Ef8, rsum, i, h)
            # 3. remaining S^T rows (k >= CH; the head's ran early), with
            # rhs_ctx rows (C2) lagging CH behind: C2(k') needs complete Ef8
            # (step 2) + colsum[k'] whose ST rows are long done.
            for k in range(min(CH, RC), RC):
                for h in range(NH):
                    score_half(T2D, T1D, ETf8, csum, k, h)
                if phases != "sonly":
                    ctx_row(Ef8, lhsb8, csum, out_rhs, LC, k - CH,
                            psC if k % 2 else psS, "c2")
                    pop_concat()
            if phases == "sonly":
                continue
            for k in range(max(0, RC - CH), RC):
                ctx_row(Ef8, lhsb8, csum, out_rhs, LC, k,
                        psC if k % 2 else psS, "c2")
                pop_concat()
            # 4. lhs_ctx rows (C1) close
            for i in range(LC):
                ctx_row(ETf8, rhsb8, rsum, out_lhs, RC, i,
                        psC if i % 2 else psS, "c1")
                pop_concat()
            while concats:
                pop_concat()

    nc.compile()
    return nc


_program = None


def _get_program():
    global _program
    if _program is None:
        _program = build_program(L, R, D)
    return _program


def kernel(lhs, rhs, w_lhs, w_rhs):
    from concourse.bass_utils import run_bass_kernel_spmd

    lhs = np.asarray(lhs, dtype=np.float32)
    rhs = np.asarray(rhs, dtype=np.float32)
    wl = np.asarray(w_lhs, dtype=np.float32).reshape(1, D)
    wr = np.asarray(w_rhs, dtype=np.float32).reshape(1, D)

    nc = _get_program()
    in_maps = [
        {"lhs": np.ascontiguousarray(lhs[c]), "rhs": np.ascontiguousarray(rhs[c]),
         "w_lhs": wl, "w_rhs": wr}
        for c in range(N_CORES)
    ]
    res = run_bass_kernel_spmd(nc, in_maps, core_ids=list(range(N_CORES)))
    out_lhs = np.stack([res.results[c]["out_lhs"] for c in range(N_CORES)])
    out_rhs = np.stack([res.results[c]["out_rhs"] for c in range(N_CORES)])
    return out_lhs, out_rhs


# revision 34
# speedup vs baseline: 5.6005x; 5.6005x over previous
"""Bidirectional attention kernel for Trainium2 (Bass/Tile), SPMD over 8 cores.

Per batch n (one batch per core):
    scores  = (lhs * w_lhs) @ (rhs * w_rhs).T          # [L, R]
            = (lhs * (w_lhs*w_rhs)) @ rhs.T            # diagonal scales compose
    E       = exp(scores)                              # no max-subtraction:
                                                       # |scores| < ~0.5 here
    lhs_ctx = (E @ rhs)   / rowsum(E)                  # row softmax folded into scale
    rhs_ctx = (E.T @ lhs) / colsum(E)                  # col softmax folded into scale
    out_lhs = [lhs | lhs_ctx],  out_rhs = [rhs | rhs_ctx]

Dual-S design: S^T is computed directly on the tensor engine from the same
resident fp8 operands instead of XBAR-transposing exp(S) (+33% PE work,
which has headroom; kills the serial Exp->transpose->cast chain and 8 MB of
SBUF<->SBUF DMA). Exp writes fp8 E and E^T directly; its accum_out yields
the row/col sums for free. All matmuls are fp8 DoubleRow (contraction
256/instr, FD=512): 1024 MMs/core, ~140us PE floor.

fp8 range: T1 carries 16*(w_lhs*w_rhs) to lift lhs*w2 (~1e-2) out of the
fp8e4 subnormal zone; the Exp activation applies scale=1/16 to compensate.

Asymmetric streaming schedule (per-engine queues execute IN ORDER, so the
emission order is the schedule):
1. lhs streams first (bf16 cast-DMA loads -> transpose -> fp8 casts),
   ~30us with PE idle -- the unavoidable head.
2. rhs streams; as chunk k lands, the FULL S^T row k runs (its moving
   operand, all of T1, is ready) -- PE works through the entire rhs
   stream. First half of rhs loads as f32 so the raw out_rhs half can be
   written from SBUF inside this window's DMA slack.
3. Natural-S row i + lhs_ctx row i (C1) interleave: C1 needs complete E^T
   (step 2) + rowsum(i) which natural row i just produced.
4. rhs_ctx rows (C2) close; DRAM->DRAM raw-half concats (lhs + second
   half of rhs) spread through steps 3-4 on the then-idle SWDGE queue.

E layouts are pair-blocked [tok%128, chunk, pair, 2, tok%128] so ctx
stationaries are contiguous 256B DoubleRow pair-blocks."""

import os
import sys

import numpy as np

for _p in ("/root/.axon_site/_ro/trn_rl_repo", "/opt/trn_rl_repo"):
    if os.path.isdir(_p) and _p not in sys.path:
        sys.path.append(_p)

N_CORES = 8
L, R, D = 2048, 2048, 1024


def build_program(L, R, D, repeat=1, phases="all"):
    from contextlib import ExitStack

    import concourse.bass as bass
    import concourse.mybir as mybir
    import concourse.tile as tile
    from concourse import bacc

    f32 = mybir.dt.float32
    bf16 = mybir.dt.bfloat16
    f8 = mybir.dt.float8e4
    DR = mybir.MatmulPerfMode.DoubleRow
    P = 128
    LC, RC, DC = L // P, R // P, D // P
    assert DC % 2 == 0 and RC % 2 == 0 and LC % 2 == 0
    QW = min(512, R)        # moving-operand width per matmul (scores FD)
    EW = min(1024, R)       # Exp width (psum tile, 2 banks)
    NH = R // EW            # Exp tiles per chunk-row
    EC = EW // P            # chunks per Exp tile
    QH = EW // QW           # QW-tiles per Exp tile
    MN = min(512, D)        # ctx matmul moving free width
    NMN = D // MN
    RSB = 0                 # all raw halves go DRAM->DRAM (keeps the rin f32
                            # path out of the stream; SBUF goes to staging)
    CH = min(8, LC)         # chunks per transpose group (amortizes the
                            # transpose-vs-DMA serialization window)

    nc = bacc.Bacc("TRN2", target_bir_lowering=False, debug=False)

    lhs = nc.dram_tensor("lhs", [L, D], f32, kind="ExternalInput")
    rhs = nc.dram_tensor("rhs", [R, D], f32, kind="ExternalInput")
    w_lhs = nc.dram_tensor("w_lhs", [1, D], f32, kind="ExternalInput")
    w_rhs = nc.dram_tensor("w_rhs", [1, D], f32, kind="ExternalInput")
    out_lhs = nc.dram_tensor("out_lhs", [L, 2 * D], f32, kind="ExternalOutput")
    out_rhs = nc.dram_tensor("out_rhs", [R, 2 * D], f32, kind="ExternalOutput")

    Exp = mybir.ActivationFunctionType.Exp
    Copy = mybir.ActivationFunctionType.Copy
    mult = mybir.AluOpType.mult
    add = mybir.AluOpType.add

    with tile.TileContext(nc) as tc, ExitStack() as ctx:
        const = ctx.enter_context(tc.tile_pool(name="const", bufs=1))
        res = ctx.enter_context(tc.tile_pool(name="res", bufs=1))
        tbp = ctx.enter_context(tc.tile_pool(name="tbp", bufs=2))
        ttp = ctx.enter_context(tc.tile_pool(name="ttp", bufs=2))
        outp = ctx.enter_context(tc.tile_pool(name="outp", bufs=3))
        scal = ctx.enter_context(tc.tile_pool(name="scal", bufs=4))

        # Resident fp8 transposed operands, [d%128, d//128, tok]: moving
        # slices merge to 3D [128, 2, QW]; stationary slices are [128, 2,
        # 128] with pair stride L.
        T1D = res.tile([P, DC, L], f8, tag="T1D")
        T2D = res.tile([P, DC, R], f8, tag="T2D")
        # E matrices, pair-blocked so ctx stationaries are contiguous:
        # Ef8[l%128, r_chunk, l_pair, l_parity, r%128] = exp(S)[l, r]
        Ef8 = res.tile([P, RC, LC // 2, 2, P], f8, tag="Ef8")
        ETf8 = res.tile([P, LC, RC // 2, 2, P], f8, tag="ETf8")
        lhsb8 = res.tile([P, LC, D], f8, tag="lhsb8")
        rhsb8 = res.tile([P, RC, D], f8, tag="rhsb8")
        rsum = res.tile([P, LC, NH], f32, tag="rsum")
        csum = res.tile([P, RC, NH], f32, tag="csum")

        psS = ctx.enter_context(tc.tile_pool(name="psS", bufs=2, space="PSUM"))
        psC = ctx.enter_context(tc.tile_pool(name="psC", bufs=2, space="PSUM"))

        for rep in range(repeat):
            # w2 = 16 * w_lhs * w_rhs in [d%128, d//128] layout
            wlT = const.tile([P, DC], f32, tag="wlT")
            wrT = const.tile([P, DC], f32, tag="wrT")
            w2T = const.tile([P, DC], f32, tag="w2T")
            # scatter-gather layout loads (1024 tiny descriptors each) go on
            # the ACT HWDGE ring, which is idle until the first Exp -- on the
            # Pool queue they would head-of-line block every input load.
            nc.scalar.dma_start(wlT[:], w_lhs[0, :].rearrange("(dc di) -> di dc", di=P))
            nc.scalar.dma_start(wrT[:], w_rhs[0, :].rearrange("(dc di) -> di dc", di=P))

            def emit_w2():
                # emitted AFTER the lhs stream: the gathers above finish
                # under it, so nothing ever waits on w2T at a queue head
                nc.vector.tensor_mul(w2T[:], wlT[:], wrT[:])
                nc.scalar.activation(w2T[:], w2T[:], Copy, scale=16.0)

            TW = min(2, CH)     # chunks per transpose (2D [128, TW*D] source,
                                # the baseline-proven wide shape)

            def prep_l_loads(g):
                c0 = g * CH
                tb4 = tbp.tile([P, CH * D], bf16, tag="tb4", name=f"tb4_l{g}")
                for j in range(CH):
                    c = c0 + j
                    nc.gpsimd.dma_start(tb4[:, j * D:(j + 1) * D],
                                        lhs[c * P:(c + 1) * P, :])
                return tb4

            def prep_l_pair_casts(g, tb4, tT4, j0, emit_rows):
                # transpose+casts for chunk pair (j0, j0+TW-1), then their
                # natural-S h=0 rows -- per-pair granularity so PE unlocks
                # row i as soon as lhs chunk i is prepped
                c0 = g * CH
                nc.sync.dma_start_transpose(
                    tT4[:, j0:j0 + TW, :, :], tb4[:, j0 * D:(j0 + TW) * D])
                w2b = w2T[:, :, None].to_broadcast((P, DC, P))
                for j in range(j0, j0 + TW):
                    c = c0 + j
                    nc.vector.tensor_tensor(T1D[:, :, c * P:(c + 1) * P],
                                            tT4[:, j, :, :], w2b, mult)
                    nc.vector.tensor_copy(lhsb8[:, c, :], tb4[:, j * D:(j + 1) * D])
                if emit_rows:
                    for j in range(j0, j0 + TW):
                        score_half(T1D, T2D, Ef8, rsum, c0 + j, 0)

            def prep_l_group(g, emit_rows=False):
                tb4 = prep_l_loads(g)
                tT4 = ttp.tile([P, CH, DC, P], bf16, tag="tT4", name=f"tT4_l{g}")
                for j0 in range(0, CH, TW):
                    prep_l_pair_casts(g, tb4, tT4, j0, emit_rows)

            def prep_r_group(g):
                c0 = g * CH
                tb4 = tbp.tile([P, CH * D], bf16, tag="tb4", name=f"tb4_r{g}")
                for j in range(CH):
                    c = c0 + j
                    nc.gpsimd.dma_start(tb4[:, j * D:(j + 1) * D],
                                        rhs[c * P:(c + 1) * P, :])
                tT4 = ttp.tile([P, CH, DC, P], bf16, tag="tT4", name=f"tT4_r{g}")
                for j in range(0, CH, TW):
                    nc.sync.dma_start_transpose(
                        tT4[:, j:j + TW, :, :], tb4[:, j * D:(j + TW) * D])
                for j in range(CH):
                    c = c0 + j
                    nc.vector.tensor_copy(T2D[:, :, c * P:(c + 1) * P],
                                          tT4[:, j, :, :])
                    nc.vector.tensor_copy(rhsb8[:, c, :], tb4[:, j * D:(j + 1) * D])

            def score_half(stat_D, mov_D, out_e, out_sum, c, h):
                """Half-row (c, h) of scores + Exp into the blocked E layout."""
                ps = psS.tile([P, EW], f32, tag="psS", name=f"ps_{c}_{h}")
                for dcp in range(0, DC, 2):
                    for q in range(QH):
                        t = h * QH + q
                        nc.tensor.matmul(
                            ps[:, q * QW:(q + 1) * QW],
                            stat_D[:, dcp:dcp + 2, c * P:(c + 1) * P],
                            mov_D[:, dcp:dcp + 2, t * QW:(t + 1) * QW],
                            start=(dcp == 0), stop=(dcp == DC - 2),
                            perf_mode=DR,
                        )
                nc.scalar.activation(
                    out_e[:, h * EC:(h + 1) * EC, c // 2, c % 2, :],
                    ps[:], Exp, scale=0.0625,
                    accum_out=out_sum[:, c, h:h + 1],
                )

            def ctx_row(stat_e, mov_nat, sums, out, CN, c, pool, lbl):
                """ctx row c: (E-slice @ mov) / sum -> out[c-chunk, D:2D]."""
                pc = pool.tile([P, D], f32, tag=pool.name, name=f"pc_{lbl}_{c}")
                for kp in range(0, CN, 2):
                    for q in range(NMN):
                        nc.tensor.matmul(
                            pc[:, q * MN:(q + 1) * MN],
                            stat_e[:, c, kp // 2, :, :],
                            mov_nat[:, kp:kp + 2, q * MN:(q + 1) * MN],
                            start=(kp == 0), stop=(kp == CN - 2),
                            perf_mode=DR,
                        )
                tot = scal.tile([P, 1], f32, tag="tot", name=f"tot_{lbl}{c}")
                rec = scal.tile([P, 1], f32, tag="rec", name=f"rec_{lbl}{c}")
                nc.vector.tensor_reduce(tot[:], sums[:, c, :], mybir.AxisListType.X, add)
                nc.vector.reciprocal(rec[:], tot[:])
                co = outp.tile([P, D], f32, tag="ctxo", name=f"co_{lbl}{c}")
                nc.vector.tensor_scalar_mul(co[:], pc[:], rec[:])
                nc.sync.dma_start(out[c * P:(c + 1) * P, D:2 * D], co[:])

            # DRAM->DRAM raw-half concats (all lhs + second half of rhs),
            # spread one per back-phase row on the then-idle SWDGE queue.
            concats = [(out_lhs, lhs, c) for c in range(LC)] + [
                (out_rhs, rhs, c) for c in range(RSB, RC)
            ]

            def pop_concat():
                # on the SP ring: the ctx-write FIFO ahead of each concat
                # paces it into the back phases (on the dep-free Pool queue
                # they would all fire during the input streams)
                if concats:
                    out, src, c = concats.pop(0)
                    nc.sync.dma_start(
                        out[c * P:(c + 1) * P, 0:D], src[c * P:(c + 1) * P, :]
                    )

            # 1. rhs head (chunks 0..EC-1: everything natural-h0 reads),
            # then lhs streams in transpose-pairs, each unlocking its
            # natural-S h=0 rows -- PE starts after ~EC+2 chunk loads.
            NLG, NRG = LC // CH, RC // CH
            RHEAD = max(1, EC // CH)
            for g in range(RHEAD):
                prep_r_group(g)
            emit_w2()
            for g in range(NLG):
                prep_l_group(g, emit_rows=True)
                if g == 0:
                    for rg in range(RHEAD, NRG):
                        prep_r_group(rg)
            # 2. natural-S h=1 rows (rhs tail is in by now)
            for h in range(1, NH):
                for i in range(LC):
                    score_half(T1D, T2D, Ef8, rsum, i, h)
            # 3. S^T rows, with rhs_ctx row k-1 (C2) interleaved: C2 needs
            # complete Ef8 (step 2) + colsum[k-1] from the ST(k-1) Exps.
            for k in range(RC):
                for h in range(NH):
                    score_half(T2D, T1D, ETf8, csum, k, h)
                if phases != "sonly" and k >= 1:
                    ctx_row(Ef8, lhsb8, csum, out_rhs, LC, k - 1,
                            psC if k % 2 else psS, "c2")
                    pop_concat()
            if phases == "sonly":
                continue
            ctx_row(Ef8, lhsb8, csum, out_rhs, LC, RC - 1, psC, "c2")
            # 4. lhs_ctx rows (C1) close
            for i in range(LC):
                ctx_row(ETf8, rhsb8, rsum, out_lhs, RC, i,
                        psC if i % 2 else psS, "c1")
                pop_concat()
            while concats:
                pop_concat()

    nc.compile()
    return nc


_program = None


def _get_program():
    global _program
    if _program is None:
        _program = build_program(L, R, D)
    return _program


def kernel(lhs, rhs, w_lhs, w_rhs):
    from concourse.bass_utils import run_bass_kernel_spmd

    lhs = np.asarray(lhs, dtype=np.float32)
    rhs = np.asarray(rhs, dtype=np.float32)
    wl = np.asarray(w_lhs, dtype=np.float32).reshape(1, D)
    wr = np.asarray(w_rhs, dtype=np.float32).reshape(1, D)

    nc = _get_program()
    in_maps = [
        {"lhs": np.ascontiguousarray(lhs[c]), "rhs": np.ascontiguousarray(rhs[c]),
         "w_lhs": wl, "w_rhs": wr}
        for c in range(N_CORES)
    ]
    res = run_bass_kernel_spmd(nc, in_maps, core_ids=list(range(N_CORES)))
    out_lhs = np.stack([res.results[c]["out_lhs"] for c in range(N_CORES)])
    out_rhs = np.stack([res.results[c]["out_rhs"] for c in range(N_CORES)])
    return out_lhs, out_rhs


# revision 35
# speedup vs baseline: 27.8388x; 4.9707x over previous
"""Bidirectional attention kernel for Trainium2 (Bass/Tile), SPMD over 8 cores.

Per batch n (one batch per core):
    scores  = (lhs * w_lhs) @ (rhs * w_rhs).T          # [L, R]
            = (lhs * (w_lhs*w_rhs)) @ rhs.T            # diagonal scales compose
    E       = exp(scores)                              # no max-subtraction:
                                                       # |scores| < ~0.5 here
    lhs_ctx = (E @ rhs)   / rowsum(E)                  # row softmax folded into scale
    rhs_ctx = (E.T @ lhs) / colsum(E)                  # col softmax folded into scale
    out_lhs = [lhs | lhs_ctx],  out_rhs = [rhs | rhs_ctx]

Dual-S design: S^T is computed directly on the tensor engine from the same
resident fp8 operands instead of XBAR-transposing exp(S) (+33% PE work,
which has headroom; kills the serial Exp->transpose->cast chain and 8 MB of
SBUF<->SBUF DMA). Exp writes fp8 E and E^T directly; its accum_out yields
the row/col sums for free. All matmuls are fp8 DoubleRow (contraction
256/instr, FD=512): 1024 MMs/core, ~140us PE floor.

fp8 range: T1 carries 16*(w_lhs*w_rhs) to lift lhs*w2 (~1e-2) out of the
fp8e4 subnormal zone; the Exp activation applies scale=1/16 to compensate.

Asymmetric streaming schedule (per-engine queues execute IN ORDER, so the
emission order is the schedule):
1. lhs streams first (bf16 cast-DMA loads -> transpose -> fp8 casts),
   ~30us with PE idle -- the unavoidable head.
2. rhs streams; as chunk k lands, the FULL S^T row k runs (its moving
   operand, all of T1, is ready) -- PE works through the entire rhs
   stream. First half of rhs loads as f32 so the raw out_rhs half can be
   written from SBUF inside this window's DMA slack.
3. Natural-S row i + lhs_ctx row i (C1) interleave: C1 needs complete E^T
   (step 2) + rowsum(i) which natural row i just produced.
4. rhs_ctx rows (C2) close; DRAM->DRAM raw-half concats (lhs + second
   half of rhs) spread through steps 3-4 on the then-idle SWDGE queue.

E layouts are pair-blocked [tok%128, chunk, pair, 2, tok%128] so ctx
stationaries are contiguous 256B DoubleRow pair-blocks."""

import os
import sys

import numpy as np

for _p in ("/root/.axon_site/_ro/trn_rl_repo", "/opt/trn_rl_repo"):
    if os.path.isdir(_p) and _p not in sys.path:
        sys.path.append(_p)

N_CORES = 8
L, R, D = 2048, 2048, 1024


def build_program(L, R, D, repeat=1, phases="all"):
    from contextlib import ExitStack

    import concourse.bass as bass
    import concourse.mybir as mybir
    import concourse.tile as tile
    from concourse import bacc

    f32 = mybir.dt.float32
    bf16 = mybir.dt.bfloat16
    f8 = mybir.dt.float8e4
    DR = mybir.MatmulPerfMode.DoubleRow
    P = 128
    LC, RC, DC = L // P, R // P, D // P
    assert DC % 2 == 0 and RC % 2 == 0 and LC % 2 == 0
    QW = min(512, R)        # moving-operand width per matmul (scores FD)
    EW = min(1024, R)       # Exp width (psum tile, 2 banks)
    NH = R // EW            # Exp tiles per chunk-row
    EC = EW // P            # chunks per Exp tile
    QH = EW // QW           # QW-tiles per Exp tile
    MN = min(512, D)        # ctx matmul moving free width
    NMN = D // MN
    RSB = 0                 # all raw halves go DRAM->DRAM (keeps the rin f32
                            # path out of the stream; SBUF goes to staging)
    CH = min(8, LC)         # chunks per transpose group (amortizes the
                            # transpose-vs-DMA serialization window)

    nc = bacc.Bacc("TRN2", target_bir_lowering=False, debug=False)

    lhs = nc.dram_tensor("lhs", [L, D], f32, kind="ExternalInput")
    rhs = nc.dram_tensor("rhs", [R, D], f32, kind="ExternalInput")
    w_lhs = nc.dram_tensor("w_lhs", [1, D], f32, kind="ExternalInput")
    w_rhs = nc.dram_tensor("w_rhs", [1, D], f32, kind="ExternalInput")
    out_lhs = nc.dram_tensor("out_lhs", [L, 2 * D], f32, kind="ExternalOutput")
    out_rhs = nc.dram_tensor("out_rhs", [R, 2 * D], f32, kind="ExternalOutput")

    Exp = mybir.ActivationFunctionType.Exp
    Copy = mybir.ActivationFunctionType.Copy
    mult = mybir.AluOpType.mult
    add = mybir.AluOpType.add

    with tile.TileContext(nc) as tc, ExitStack() as ctx:
        const = ctx.enter_context(tc.tile_pool(name="const", bufs=1))
        res = ctx.enter_context(tc.tile_pool(name="res", bufs=1))
        tbp = ctx.enter_context(tc.tile_pool(name="tbp", bufs=2))
        ttp = ctx.enter_context(tc.tile_pool(name="ttp", bufs=2))
        outp = ctx.enter_context(tc.tile_pool(name="outp", bufs=3))
        scal = ctx.enter_context(tc.tile_pool(name="scal", bufs=4))

        # Resident fp8 transposed operands, [d%128, d//128, tok]: moving
        # slices merge to 3D [128, 2, QW]; stationary slices are [128, 2,
        # 128] with pair stride L.
        T1D = res.tile([P, DC, L], f8, tag="T1D")
        T2D = res.tile([P, DC, R], f8, tag="T2D")
        # E matrices, pair-blocked so ctx stationaries are contiguous:
        # Ef8[l%128, r_chunk, l_pair, l_parity, r%128] = exp(S)[l, r]
        Ef8 = res.tile([P, RC, LC // 2, 2, P], f8, tag="Ef8")
        ETf8 = res.tile([P, LC, RC // 2, 2, P], f8, tag="ETf8")
        lhsb8 = res.tile([P, LC, D], f8, tag="lhsb8")
        rhsb8 = res.tile([P, RC, D], f8, tag="rhsb8")
        rsum = res.tile([P, LC, NH], f32, tag="rsum")
        csum = res.tile([P, RC, NH], f32, tag="csum")

        psS = ctx.enter_context(tc.tile_pool(name="psS", bufs=2, space="PSUM"))
        psC = ctx.enter_context(tc.tile_pool(name="psC", bufs=2, space="PSUM"))

        for rep in range(repeat):
            # w2 = 16 * w_lhs * w_rhs in [d%128, d//128] layout
            wlT = const.tile([P, DC], f32, tag="wlT")
            wrT = const.tile([P, DC], f32, tag="wrT")
            w2T = const.tile([P, DC], f32, tag="w2T")
            # w layout loads as DC small partition-spray DMAs (512B
            # contiguous each) on the ACT HWDGE ring -- a single rearranged
            # gather would be 1024 tiny descriptors occupying the DMA device
            # for ~10us, which every XBAR transpose serializes against.
            for wt, wsrc in ((wlT, w_lhs), (wrT, w_rhs)):
                for dc in range(DC):
                    nc.scalar.dma_start(
                        wt[:, dc:dc + 1],
                        wsrc[0, dc * P:(dc + 1) * P].rearrange(
                            "(di one) -> di one", one=1),
                    )

            def emit_w2():
                # emitted AFTER the lhs stream: the gathers above finish
                # under it, so nothing ever waits on w2T at a queue head
                nc.vector.tensor_mul(w2T[:], wlT[:], wrT[:])
                nc.scalar.activation(w2T[:], w2T[:], Copy, scale=16.0)

            TW = min(2, CH)     # chunks per transpose (2D [128, TW*D] source,
                                # the baseline-proven wide shape)

            def prep_l_loads(g):
                c0 = g * CH
                tb4 = tbp.tile([P, CH * D], bf16, tag="tb4", name=f"tb4_l{g}")
                for j in range(CH):
                    c = c0 + j
                    nc.gpsimd.dma_start(tb4[:, j * D:(j + 1) * D],
                                        lhs[c * P:(c + 1) * P, :])
                return tb4

            def prep_l_pair_casts(g, tb4, tT4, j0, emit_rows):
                # transpose+casts for chunk pair (j0, j0+TW-1), then their
                # natural-S h=0 rows -- per-pair granularity so PE unlocks
                # row i as soon as lhs chunk i is prepped
                c0 = g * CH
                nc.sync.dma_start_transpose(
                    tT4[:, j0:j0 + TW, :, :], tb4[:, j0 * D:(j0 + TW) * D])
                w2b = w2T[:, :, None].to_broadcast((P, DC, P))
                for j in range(j0, j0 + TW):
                    c = c0 + j
                    nc.vector.tensor_tensor(T1D[:, :, c * P:(c + 1) * P],
                                            tT4[:, j, :, :], w2b, mult)
                    nc.vector.tensor_copy(lhsb8[:, c, :], tb4[:, j * D:(j + 1) * D])
                if emit_rows:
                    for j in range(j0, j0 + TW):
                        score_half(T1D, T2D, Ef8, rsum, c0 + j, 0)

            def prep_l_group(g, emit_rows=False):
                tb4 = prep_l_loads(g)
                tT4 = ttp.tile([P, CH, DC, P], bf16, tag="tT4", name=f"tT4_l{g}")
                for j0 in range(0, CH, TW):
                    prep_l_pair_casts(g, tb4, tT4, j0, emit_rows)

            def prep_r_group(g):
                c0 = g * CH
                tb4 = tbp.tile([P, CH * D], bf16, tag="tb4", name=f"tb4_r{g}")
                for j in range(CH):
                    c = c0 + j
                    nc.gpsimd.dma_start(tb4[:, j * D:(j + 1) * D],
                                        rhs[c * P:(c + 1) * P, :])
                tT4 = ttp.tile([P, CH, DC, P], bf16, tag="tT4", name=f"tT4_r{g}")
                for j in range(0, CH, TW):
                    nc.sync.dma_start_transpose(
                        tT4[:, j:j + TW, :, :], tb4[:, j * D:(j + TW) * D])
                for j in range(CH):
                    c = c0 + j
                    nc.vector.tensor_copy(T2D[:, :, c * P:(c + 1) * P],
                                          tT4[:, j, :, :])
                    nc.vector.tensor_copy(rhsb8[:, c, :], tb4[:, j * D:(j + 1) * D])

            def score_half(stat_D, mov_D, out_e, out_sum, c, h):
                """Half-row (c, h) of scores + Exp into the blocked E layout."""
                ps = psS.tile([P, EW], f32, tag="psS", name=f"ps_{c}_{h}")
                for dcp in range(0, DC, 2):
                    for q in range(QH):
                        t = h * QH + q
                        nc.tensor.matmul(
                            ps[:, q * QW:(q + 1) * QW],
                            stat_D[:, dcp:dcp + 2, c * P:(c + 1) * P],
                            mov_D[:, dcp:dcp + 2, t * QW:(t + 1) * QW],
                            start=(dcp == 0), stop=(dcp == DC - 2),
                            perf_mode=DR,
                        )
                nc.scalar.activation(
                    out_e[:, h * EC:(h + 1) * EC, c // 2, c % 2, :],
                    ps[:], Exp, scale=0.0625,
                    accum_out=out_sum[:, c, h:h + 1],
                )

            def ctx_row(stat_e, mov_nat, sums, out, CN, c, pool, lbl):
                """ctx row c: (E-slice @ mov) / sum -> out[c-chunk, D:2D]."""
                pc = pool.tile([P, D], f32, tag=pool.name, name=f"pc_{lbl}_{c}")
                for kp in range(0, CN, 2):
                    for q in range(NMN):
                        nc.tensor.matmul(
                            pc[:, q * MN:(q + 1) * MN],
                            stat_e[:, c, kp // 2, :, :],
                            mov_nat[:, kp:kp + 2, q * MN:(q + 1) * MN],
                            start=(kp == 0), stop=(kp == CN - 2),
                            perf_mode=DR,
                        )
                tot = scal.tile([P, 1], f32, tag="tot", name=f"tot_{lbl}{c}")
                rec = scal.tile([P, 1], f32, tag="rec", name=f"rec_{lbl}{c}")
                nc.vector.tensor_reduce(tot[:], sums[:, c, :], mybir.AxisListType.X, add)
                nc.vector.reciprocal(rec[:], tot[:])
                co = outp.tile([P, D], f32, tag="ctxo", name=f"co_{lbl}{c}")
                nc.vector.tensor_scalar_mul(co[:], pc[:], rec[:])
                nc.sync.dma_start(out[c * P:(c + 1) * P, D:2 * D], co[:])

            # DRAM->DRAM raw-half concats (all lhs + second half of rhs),
            # spread one per back-phase row on the then-idle SWDGE queue.
            concats = [(out_lhs, lhs, c) for c in range(LC)] + [
                (out_rhs, rhs, c) for c in range(RSB, RC)
            ]

            def pop_concat():
                # on the SP ring: the ctx-write FIFO ahead of each concat
                # paces it into the back phases (on the dep-free Pool queue
                # they would all fire during the input streams)
                if concats:
                    out, src, c = concats.pop(0)
                    nc.sync.dma_start(
                        out[c * P:(c + 1) * P, 0:D], src[c * P:(c + 1) * P, :]
                    )

            # 1. rhs head (chunks 0..EC-1: everything natural-h0 reads),
            # then lhs streams in transpose-pairs, each unlocking its
            # natural-S h=0 rows -- PE starts after ~EC+2 chunk loads.
            NLG, NRG = LC // CH, RC // CH
            RHEAD = max(1, EC // CH)
            for g in range(RHEAD):
                prep_r_group(g)
            emit_w2()
            for g in range(NLG):
                prep_l_group(g, emit_rows=True)
                if g == 0:
                    for rg in range(RHEAD, NRG):
                        prep_r_group(rg)
            # 2. natural-S h=1 rows (rhs tail is in by now)
            for h in range(1, NH):
                for i in range(LC):
                    score_half(T1D, T2D, Ef8, rsum, i, h)
            # 3. S^T rows, with rhs_ctx row k-1 (C2) interleaved: C2 needs
            # complete Ef8 (step 2) + colsum[k-1] from the ST(k-1) Exps.
            for k in range(RC):
                for h in range(NH):
                    score_half(T2D, T1D, ETf8, csum, k, h)
                if phases != "sonly" and k >= 1:
                    ctx_row(Ef8, lhsb8, csum, out_rhs, LC, k - 1,
                            psC if k % 2 else psS, "c2")
                    pop_concat()
            if phases == "sonly":
                continue
            ctx_row(Ef8, lhsb8, csum, out_rhs, LC, RC - 1, psC, "c2")
            # 4. lhs_ctx rows (C1) close
            for i in range(LC):
                ctx_row(ETf8, rhsb8, rsum, out_lhs, RC, i,
                        psC if i % 2 else psS, "c1")
                pop_concat()
            while concats:
                pop_concat()

    nc.compile()
    return nc


_program = None


def _get_program():
    global _program
    if _program is None:
        _program = build_program(L, R, D)
    return _program


def kernel(lhs, rhs, w_lhs, w_rhs):
    from concourse.bass_utils import run_bass_kernel_spmd

    lhs = np.asarray(lhs, dtype=np.float32)
    rhs = np.asarray(rhs, dtype=np.float32)
    wl = np.asarray(w_lhs, dtype=np.float32).reshape(1, D)
    wr = np.asarray(w_rhs, dtype=np.float32).reshape(1, D)

    nc = _get_program()
    in_maps = [
        {"lhs": np.ascontiguousarray(lhs[c]), "rhs": np.ascontiguousarray(rhs[c]),
         "w_lhs": wl, "w_rhs": wr}
        for c in range(N_CORES)
    ]
    res = run_bass_kernel_spmd(nc, in_maps, core_ids=list(range(N_CORES)))
    out_lhs = np.stack([res.results[c]["out_lhs"] for c in range(N_CORES)])
    out_rhs = np.stack([res.results[c]["out_rhs"] for c in range(N_CORES)])
    return out_lhs, out_rhs


# revision 36
# speedup vs baseline: 61.7747x; 2.2190x over previous
"""Bidirectional attention kernel for Trainium2 (Bass/Tile), SPMD over 8 cores.

Per batch n (one batch per core):
    scores  = (lhs * w_lhs) @ (rhs * w_rhs).T          # [L, R]
            = (lhs * (w_lhs*w_rhs)) @ rhs.T            # diagonal scales compose
    E       = exp(scores)                              # no max-subtraction:
                                                       # |scores| < ~0.5 here
    lhs_ctx = (E @ rhs)   / rowsum(E)                  # row softmax folded into scale
    rhs_ctx = (E.T @ lhs) / colsum(E)                  # col softmax folded into scale
    out_lhs = [lhs | lhs_ctx],  out_rhs = [rhs | rhs_ctx]

Dual-S design: S^T is computed directly on the tensor engine from the same
resident fp8 operands instead of XBAR-transposing exp(S) (+33% PE work,
which has headroom; kills the serial Exp->transpose->cast chain and 8 MB of
SBUF<->SBUF DMA). Exp writes fp8 E and E^T directly; its accum_out yields
the row/col sums for free. All matmuls are fp8 DoubleRow (contraction
256/instr, FD=512): 1024 MMs/core, ~140us PE floor.

fp8 range: T1 carries 16*(w_lhs*w_rhs) to lift lhs*w2 (~1e-2) out of the
fp8e4 subnormal zone; the Exp activation applies scale=1/16 to compensate.

Asymmetric streaming schedule (per-engine queues execute IN ORDER, so the
emission order is the schedule):
1. lhs streams first (bf16 cast-DMA loads -> transpose -> fp8 casts),
   ~30us with PE idle -- the unavoidable head.
2. rhs streams; as chunk k lands, the FULL S^T row k runs (its moving
   operand, all of T1, is ready) -- PE works through the entire rhs
   stream. First half of rhs loads as f32 so the raw out_rhs half can be
   written from SBUF inside this window's DMA slack.
3. Natural-S row i + lhs_ctx row i (C1) interleave: C1 needs complete E^T
   (step 2) + rowsum(i) which natural row i just produced.
4. rhs_ctx rows (C2) close; DRAM->DRAM raw-half concats (lhs + second
   half of rhs) spread through steps 3-4 on the then-idle SWDGE queue.

E layouts are pair-blocked [tok%128, chunk, pair, 2, tok%128] so ctx
stationaries are contiguous 256B DoubleRow pair-blocks."""

import os
import sys

import numpy as np

for _p in ("/root/.axon_site/_ro/trn_rl_repo", "/opt/trn_rl_repo"):
    if os.path.isdir(_p) and _p not in sys.path:
        sys.path.append(_p)

N_CORES = 8
L, R, D = 2048, 2048, 1024


def build_program(L, R, D, repeat=1, phases="all"):
    from contextlib import ExitStack

    import concourse.bass as bass
    import concourse.mybir as mybir
    import concourse.tile as tile
    from concourse import bacc

    f32 = mybir.dt.float32
    bf16 = mybir.dt.bfloat16
    f8 = mybir.dt.float8e4
    DR = mybir.MatmulPerfMode.DoubleRow
    P = 128
    LC, RC, DC = L // P, R // P, D // P
    assert DC % 2 == 0 and RC % 2 == 0 and LC % 2 == 0
    QW = min(512, R)        # moving-operand width per matmul (scores FD)
    EW = min(1024, R)       # Exp width (psum tile, 2 banks)
    NH = R // EW            # Exp tiles per chunk-row
    EC = EW // P            # chunks per Exp tile
    QH = EW // QW           # QW-tiles per Exp tile
    MN = min(512, D)        # ctx matmul moving free width
    NMN = D // MN
    RSB = 0                 # all raw halves go DRAM->DRAM (keeps the rin f32
                            # path out of the stream; SBUF goes to staging)
    CH = min(8, LC)         # chunks per transpose group (amortizes the
                            # transpose-vs-DMA serialization window)

    nc = bacc.Bacc("TRN2", target_bir_lowering=False, debug=False)

    lhs = nc.dram_tensor("lhs", [L, D], f32, kind="ExternalInput")
    rhs = nc.dram_tensor("rhs", [R, D], f32, kind="ExternalInput")
    w_lhs = nc.dram_tensor("w_lhs", [1, D], f32, kind="ExternalInput")
    w_rhs = nc.dram_tensor("w_rhs", [1, D], f32, kind="ExternalInput")
    out_lhs = nc.dram_tensor("out_lhs", [L, 2 * D], f32, kind="ExternalOutput")
    out_rhs = nc.dram_tensor("out_rhs", [R, 2 * D], f32, kind="ExternalOutput")

    Exp = mybir.ActivationFunctionType.Exp
    Copy = mybir.ActivationFunctionType.Copy
    mult = mybir.AluOpType.mult
    add = mybir.AluOpType.add

    with tile.TileContext(nc) as tc, ExitStack() as ctx:
        const = ctx.enter_context(tc.tile_pool(name="const", bufs=1))
        res = ctx.enter_context(tc.tile_pool(name="res", bufs=1))
        tbp = ctx.enter_context(tc.tile_pool(name="tbp", bufs=2))
        ttp = ctx.enter_context(tc.tile_pool(name="ttp", bufs=2))
        outp = ctx.enter_context(tc.tile_pool(name="outp", bufs=3))
        scal = ctx.enter_context(tc.tile_pool(name="scal", bufs=4))

        # Resident fp8 transposed operands, [d%128, d//128, tok]: moving
        # slices merge to 3D [128, 2, QW]; stationary slices are [128, 2,
        # 128] with pair stride L.
        T1D = res.tile([P, DC, L], f8, tag="T1D")
        T2D = res.tile([P, DC, R], f8, tag="T2D")
        # E matrices, pair-blocked so ctx stationaries are contiguous:
        # Ef8[l%128, r_chunk, l_pair, l_parity, r%128] = exp(S)[l, r]
        Ef8 = res.tile([P, RC, LC // 2, 2, P], f8, tag="Ef8")
        ETf8 = res.tile([P, LC, RC // 2, 2, P], f8, tag="ETf8")
        lhsb8 = res.tile([P, LC, D], f8, tag="lhsb8")
        rhsb8 = res.tile([P, RC, D], f8, tag="rhsb8")
        rsum = res.tile([P, LC, NH], f32, tag="rsum")
        csum = res.tile([P, RC, NH], f32, tag="csum")

        psS = ctx.enter_context(tc.tile_pool(name="psS", bufs=2, space="PSUM"))
        psC = ctx.enter_context(tc.tile_pool(name="psC", bufs=2, space="PSUM"))

        for rep in range(repeat):
            # w2 = 16 * w_lhs * w_rhs in [d%128, d//128] layout
            wlT = const.tile([P, DC], f32, tag="wlT")
            wrT = const.tile([P, DC], f32, tag="wrT")
            w2T = const.tile([P, DC], f32, tag="w2T")
            # w layout loads as DC small partition-spray DMAs (512B
            # contiguous each) on the ACT HWDGE ring -- a single rearranged
            # gather would be 1024 tiny descriptors occupying the DMA device
            # for ~10us, which every XBAR transpose serializes against.
            for wt, wsrc in ((wlT, w_lhs), (wrT, w_rhs)):
                for dc in range(DC):
                    nc.scalar.dma_start(
                        wt[:, dc:dc + 1],
                        wsrc[0, dc * P:(dc + 1) * P].rearrange(
                            "(di one) -> di one", one=1),
                    )

            def emit_w2():
                # emitted AFTER the lhs stream: the gathers above finish
                # under it, so nothing ever waits on w2T at a queue head
                nc.vector.tensor_mul(w2T[:], wlT[:], wrT[:])
                nc.scalar.activation(w2T[:], w2T[:], Copy, scale=16.0)

            TW = min(2, CH)     # chunks per transpose (2D [128, TW*D] source,
                                # the baseline-proven wide shape)

            def prep_l_loads(g):
                c0 = g * CH
                tb4 = tbp.tile([P, CH * D], bf16, tag="tb4", name=f"tb4_l{g}")
                for j in range(CH):
                    c = c0 + j
                    nc.gpsimd.dma_start(tb4[:, j * D:(j + 1) * D],
                                        lhs[c * P:(c + 1) * P, :])
                return tb4

            def prep_l_pair_casts(g, tb4, tT4, j0, emit_rows):
                # transpose+casts for chunk pair (j0, j0+TW-1), then their
                # natural-S h=0 rows -- per-pair granularity so PE unlocks
                # row i as soon as lhs chunk i is prepped
                c0 = g * CH
                nc.sync.dma_start_transpose(
                    tT4[:, j0:j0 + TW, :, :], tb4[:, j0 * D:(j0 + TW) * D])
                w2b = w2T[:, :, None].to_broadcast((P, DC, P))
                for j in range(j0, j0 + TW):
                    c = c0 + j
                    nc.vector.tensor_tensor(T1D[:, :, c * P:(c + 1) * P],
                                            tT4[:, j, :, :], w2b, mult)
                    nc.vector.tensor_copy(lhsb8[:, c, :], tb4[:, j * D:(j + 1) * D])
                if emit_rows:
                    for j in range(j0, j0 + TW):
                        score_half(T1D, T2D, Ef8, rsum, c0 + j, 0)

            def prep_l_group(g, emit_rows=False):
                tb4 = prep_l_loads(g)
                tT4 = ttp.tile([P, CH, DC, P], bf16, tag="tT4", name=f"tT4_l{g}")
                for j0 in range(0, CH, TW):
                    prep_l_pair_casts(g, tb4, tT4, j0, emit_rows)

            def prep_r_group(g):
                c0 = g * CH
                tb4 = tbp.tile([P, CH * D], bf16, tag="tb4", name=f"tb4_r{g}")
                for j in range(CH):
                    c = c0 + j
                    nc.gpsimd.dma_start(tb4[:, j * D:(j + 1) * D],
                                        rhs[c * P:(c + 1) * P, :])
                tT4 = ttp.tile([P, CH, DC, P], bf16, tag="tT4", name=f"tT4_r{g}")
                for j in range(0, CH, TW):
                    nc.sync.dma_start_transpose(
                        tT4[:, j:j + TW, :, :], tb4[:, j * D:(j + TW) * D])
                for j in range(CH):
                    c = c0 + j
                    nc.vector.tensor_copy(T2D[:, :, c * P:(c + 1) * P],
                                          tT4[:, j, :, :])
                    nc.vector.tensor_copy(rhsb8[:, c, :], tb4[:, j * D:(j + 1) * D])

            def score_half(stat_D, mov_D, out_e, out_sum, c, h):
                """Half-row (c, h) of scores + Exp into the blocked E layout."""
                ps = psS.tile([P, EW], f32, tag="psS", name=f"ps_{c}_{h}")
                for dcp in range(0, DC, 2):
                    for q in range(QH):
                        t = h * QH + q
                        nc.tensor.matmul(
                            ps[:, q * QW:(q + 1) * QW],
                            stat_D[:, dcp:dcp + 2, c * P:(c + 1) * P],
                            mov_D[:, dcp:dcp + 2, t * QW:(t + 1) * QW],
                            start=(dcp == 0), stop=(dcp == DC - 2),
                            perf_mode=DR,
                        )
                nc.scalar.activation(
                    out_e[:, h * EC:(h + 1) * EC, c // 2, c % 2, :],
                    ps[:], Exp, scale=0.0625,
                    accum_out=out_sum[:, c, h:h + 1],
                )

            def ctx_row(stat_e, mov_nat, sums, out, CN, c, pool, lbl):
                """ctx row c: (E-slice @ mov) / sum -> out[c-chunk, D:2D]."""
                pc = pool.tile([P, D], f32, tag=pool.name, name=f"pc_{lbl}_{c}")
                for kp in range(0, CN, 2):
                    for q in range(NMN):
                        nc.tensor.matmul(
                            pc[:, q * MN:(q + 1) * MN],
                            stat_e[:, c, kp // 2, :, :],
                            mov_nat[:, kp:kp + 2, q * MN:(q + 1) * MN],
                            start=(kp == 0), stop=(kp == CN - 2),
                            perf_mode=DR,
                        )
                tot = scal.tile([P, 1], f32, tag="tot", name=f"tot_{lbl}{c}")
                rec = scal.tile([P, 1], f32, tag="rec", name=f"rec_{lbl}{c}")
                nc.vector.tensor_reduce(tot[:], sums[:, c, :], mybir.AxisListType.X, add)
                nc.vector.reciprocal(rec[:], tot[:])
                co = outp.tile([P, D], f32, tag="ctxo", name=f"co_{lbl}{c}")
                nc.vector.tensor_scalar_mul(co[:], pc[:], rec[:])
                nc.sync.dma_start(out[c * P:(c + 1) * P, D:2 * D], co[:])

            # DRAM->DRAM raw-half concats (all lhs + second half of rhs),
            # spread one per back-phase row on the then-idle SWDGE queue.
            concats = [(out_lhs, lhs, c) for c in range(LC)] + [
                (out_rhs, rhs, c) for c in range(RSB, RC)
            ]

            def pop_concat():
                # on the SP ring: the ctx-write FIFO ahead of each concat
                # paces it into the back phases (on the dep-free Pool queue
                # they would all fire during the input streams)
                if concats:
                    out, src, c = concats.pop(0)
                    nc.sync.dma_start(
                        out[c * P:(c + 1) * P, 0:D], src[c * P:(c + 1) * P, :]
                    )

            # 1. rhs head (chunks 0..EC-1: everything natural-h0 reads),
            # then lhs streams in transpose-pairs, each unlocking its
            # natural-S h=0 rows -- PE starts after ~EC+2 chunk loads.
            NLG, NRG = LC // CH, RC // CH
            RHEAD = max(1, EC // CH)
            for g in range(RHEAD):
                prep_r_group(g)
            emit_w2()
            for g in range(NLG):
                prep_l_group(g, emit_rows=True)
                if g == 0:
                    # S^T h=0 rows of the rhs head: only need lhs group 0
                    # (T1 quarters 0-1) -- PE backlog covering the
                    # second-wave prep window. Pure PE-order change.
                    for k in range(min(CH, RC)):
                        score_half(T2D, T1D, ETf8, csum, k, 0)
                    for rg in range(RHEAD, NRG):
                        prep_r_group(rg)
                elif g == 1 and NH > 1:
                    for k in range(min(CH, RC)):
                        score_half(T2D, T1D, ETf8, csum, k, 1)
            # 2. natural-S h=1 rows (rhs tail is in by now)
            for h in range(1, NH):
                for i in range(LC):
                    score_half(T1D, T2D, Ef8, rsum, i, h)
            # 3. remaining S^T rows (the head's ran early), with rhs_ctx
            # rows (C2) lagging CH behind: C2(k') needs complete Ef8 +
            # colsum[k'] whose ST rows are long done.
            for k in range(min(CH, RC), RC):
                for h in range(NH):
                    score_half(T2D, T1D, ETf8, csum, k, h)
                if phases != "sonly":
                    ctx_row(Ef8, lhsb8, csum, out_rhs, LC, k - CH,
                            psC if k % 2 else psS, "c2")
                    pop_concat()
            if phases == "sonly":
                continue
            for k in range(max(0, RC - CH), RC):
                ctx_row(Ef8, lhsb8, csum, out_rhs, LC, k,
                        psC if k % 2 else psS, "c2")
                pop_concat()
            # 4. lhs_ctx rows (C1) close
            for i in range(LC):
                ctx_row(ETf8, rhsb8, rsum, out_lhs, RC, i,
                        psC if i % 2 else psS, "c1")
                pop_concat()
            while concats:
                pop_concat()

    nc.compile()
    return nc


_program = None


def _get_program():
    global _program
    if _program is None:
        _program = build_program(L, R, D)
    return _program


def kernel(lhs, rhs, w_lhs, w_rhs):
    from concourse.bass_utils import run_bass_kernel_spmd

    lhs = np.asarray(lhs, dtype=np.float32)
    rhs = np.asarray(rhs, dtype=np.float32)
    wl = np.asarray(w_lhs, dtype=np.float32).reshape(1, D)
    wr = np.asarray(w_rhs, dtype=np.float32).reshape(1, D)

    nc = _get_program()
    in_maps = [
        {"lhs": np.ascontiguousarray(lhs[c]), "rhs": np.ascontiguousarray(rhs[c]),
         "w_lhs": wl, "w_rhs": wr}
        for c in range(N_CORES)
    ]
    res = run_bass_kernel_spmd(nc, in_maps, core_ids=list(range(N_CORES)))
    out_lhs = np.stack([res.results[c]["out_lhs"] for c in range(N_CORES)])
    out_rhs = np.stack([res.results[c]["out_rhs"] for c in range(N_CORES)])
    return out_lhs, out_rhs
